# revision 1
# baseline (speedup 1.0000x reference)
"""Trainium2 Bass kernel for ContinuousREWAEncoder:
    out = FWHT(x @ W^T)/sqrt(32) + 0.01*normal(key=42)

Math folding: FWHT is linear => out = x @ (H @ W / sqrt(32))^T + noise.
The noise uses a fixed PRNG key, so it is a deterministic constant computed
on host (with the same jax op/backend as the reference) and streamed in.

Sharding: pure data parallel over tokens (B*N = 32768 -> 4096/core on 8
cores). W_eff is replicated. Each x shard is pre-tiled on host so the
contraction dim D lies on SBUF partitions and every DMA is one contiguous
run per partition. The device does a single streaming GEMM:
psum[32, t] += Wc[128,32]^T @ xT[128, t] accumulated over 8 d-chunks, with
the noise added during PSUM evacuation by the DVE, and the output stored
transposed [32, tok] (un-transposed on host).
"""

import math

import numpy as np

import concourse.tile as tile
from concourse import bacc, mybir
from concourse.bass_utils import run_bass_kernel_spmd

B, N, D, M = 4, 8192, 1024, 32
NOISE_STD = 0.01
N_CORES = 8
TOK_TOTAL = B * N              # 32768
TOK = TOK_TOTAL // N_CORES     # 4096 tokens per core
BLK = 512                      # tokens per PSUM bank ([32, 512] fp32 = 1 bank)
NBLK = TOK // BLK              # 8 -> exactly the 8 PSUM banks
KC = D // 128                  # 8 contraction chunks

# Matmul input dtype:
#   "fp16": half the HBM traffic (the kernel is memory-bound) and full-rate
#           PE; measured absmax rel err ~2.4e-4 vs the fp32 reference.
#   "fp32r": fp32 bits via the fast PE mode; absmax rel err ~1.2e-4.
MM_MODE = "fp16"
MM_DT = mybir.dt.float16 if MM_MODE == "fp16" else mybir.dt.float32r
MM_NP = np.float16 if MM_MODE == "fp16" else np.float32
F32 = mybir.dt.float32


def _build_bass():
    nc = bacc.Bacc("TRN2", target_bir_lowering=False)

    # x pre-tiled on host to [blk, partition, kchunk*BLK] so each DMA moves
    # one fully-contiguous run per partition (128 big descriptors -> full
    # HBM streaming rate).
    xT = nc.dram_tensor("xT", [NBLK, 128, KC * BLK], MM_DT, kind="ExternalInput")
    # w pre-packed on host to the SBUF layout [partition, kchunk*M]:
    # one contiguous run per partition keeps its DMA small and fast.
    wT = nc.dram_tensor("wT", [128, KC * M], MM_DT, kind="ExternalInput")
    nzT = nc.dram_tensor("noiseT", [M, TOK], F32, kind="ExternalInput")
    outT = nc.dram_tensor("outT", [M, TOK], F32, kind="ExternalOutput")

    with tile.TileContext(nc) as tc:
        with (
            tc.tile_pool(name="w", bufs=1) as wpool,
            tc.tile_pool(name="nz", bufs=1) as nzpool,
            tc.tile_pool(name="x", bufs=6) as xpool,
            tc.tile_pool(name="xlast", bufs=1) as xlpool,
            tc.tile_pool(name="out", bufs=4) as opool,
            tc.tile_pool(name="psum", bufs=NBLK, space="PSUM") as ppool,
        ):
            # Noise on the scalar HWDGE ring (off the x stream's ring).
            nz_tile = nzpool.tile([M, TOK], F32)
            nc.scalar.dma_start(nz_tile[:], nzT[:])

            # w on the sync ring ahead of the x stream (FIFO per ring) so
            # the warmup matmul unblocks before the first x tile lands.
            w_tile = wpool.tile([128, KC, M], MM_DT)
            nc.sync.dma_start(w_tile[:], wT.rearrange("p (c m) -> p c m", c=KC))

            x_tiles = []
            for b in range(NBLK - 1):
                t = xpool.tile([128, KC, BLK], MM_DT, tag="xt")
                nc.sync.dma_start(t[:], xT[b].rearrange("p (c t) -> p c t", c=KC))
                x_tiles.append(t)

            # Last block: chunks 0-6 in one DMA (large packets), chunk 7
            # alone. After the final 128 KB lands, only one matmul (not 8)
            # remains before the add+store, shortening the kernel tail,
            # while keeping nearly all packets at full streaming size.
            # Separate tiles keep every matmul at exactly one sync wait.
            xlast = xT[NBLK - 1].rearrange("p (c t) -> p c t", c=KC)
            xlast_a = xlpool.tile([128, KC - 1, BLK], MM_DT, tag="xla")
            nc.sync.dma_start(xlast_a[:], xlast[:, 0 : KC - 1, :])
            xlast_b = xlpool.tile([128, BLK], MM_DT, tag="xlb")
            nc.sync.dma_start(xlast_b[:], xlast[:, KC - 1, :])

            # fp32/fp16 matmuls self-load weights and their codegen struct
            # only supports a single sync wait. This warmup matmul absorbs
            # the w-DMA wait into PE program order so every real matmul
            # needs only its x-DMA wait.
            warm = ppool.tile([M, M], F32, tag="ptile")
            nc.tensor.matmul(warm[:], w_tile[:, 0, :], w_tile[:, 0, :])

            for b in range(NBLK):
                ptile = ppool.tile([M, BLK], F32, tag="ptile")
                for c in range(KC):
                    if b < NBLK - 1:
                        rhs = x_tiles[b][:, c, :]
                    elif c < KC - 1:
                        rhs = xlast_a[:, c, :]
                    else:
                        rhs = xlast_b[:]
                    nc.tensor.matmul(
                        ptile[:],
                        w_tile[:, c, :],
                        rhs,
                        start=(c == 0),
                        stop=(c == KC - 1),
                    )

                o_tile = opool.tile([M, BLK], F32)
                nc.vector.tensor_add(
                    o_tile[:], ptile[:], nz_tile[:, b * BLK : (b + 1) * BLK]
                )
                nc.scalar.dma_start(outT[:, b * BLK : (b + 1) * BLK], o_tile[:])

    nc.compile()
    return nc


_NC_CACHE = None


def _get_nc():
    global _NC_CACHE
    if _NC_CACHE is None:
        _NC_CACHE = _build_bass()
    return _NC_CACHE


def _hadamard32() -> np.ndarray:
    h = np.array([[1.0]], dtype=np.float64)
    while h.shape[0] < M:
        h = np.block([[h, h], [h, -h]])
    return h


_NOISE_CACHE = None


def _noise() -> np.ndarray:
    # Mirror reference.py exactly (same op on the default jax backend): the
    # bits differ between backends, so the noise must be produced the same
    # way the grading reference produces it.
    global _NOISE_CACHE
    if _NOISE_CACHE is None:
        import jax

        nz = NOISE_STD * jax.random.normal(
            jax.random.key(42), (B, N, M), dtype=np.float32
        )
        _NOISE_CACHE = np.asarray(nz)
    return _NOISE_CACHE


def kernel(x: np.ndarray, W: np.ndarray, _profile_sink=None) -> np.ndarray:
    x = np.ascontiguousarray(np.asarray(x, dtype=np.float32))
    W = np.asarray(W, dtype=np.float32)

    # Fold normalized FWHT into the projection: out = x @ w_lhsT + noise
    w_eff = (_hadamard32() @ W.astype(np.float64)) / math.sqrt(M)
    w_lhsT = w_eff.T.astype(MM_NP)  # [D, M]
    # pack to device SBUF layout [partition, kchunk, M]
    w_dev = np.ascontiguousarray(
        w_lhsT.reshape(KC, 128, M).transpose(1, 0, 2)
    ).reshape(128, KC * M)

    noise = _noise().reshape(TOK_TOTAL, M)
    X = x.reshape(TOK_TOTAL, D).astype(MM_NP, copy=False)

    in_maps = []
    for i in range(N_CORES):
        sl = slice(i * TOK, (i + 1) * TOK)
        # [tok, d] -> [blk, partition, kchunk, tok_in_blk] contiguous
        xt = np.ascontiguousarray(
            X[sl].reshape(NBLK, BLK, KC, 128).transpose(0, 3, 2, 1)
        ).reshape(NBLK, 128, KC * BLK)
        in_maps.append(
            {
                "xT": xt,
                "wT": w_dev,
                "noiseT": np.ascontiguousarray(noise[sl].T),
            }
        )

    res = run_bass_kernel_spmd(
        _get_nc(),
        in_maps,
        core_ids=list(range(N_CORES)),
        trace=_profile_sink is not None,
    )
    if _profile_sink is not None:
        _profile_sink.append(res)

    out = np.concatenate([r["outT"].T for r in res.results], axis=0)
    return np.ascontiguousarray(out.reshape(B, N, M).astype(np.float32))


if __name__ == "__main__":
    xs = np.random.randn(B, N, D).astype(np.float32)
    Ws = (np.random.randn(M, D) / math.sqrt(D)).astype(np.float32)
    o = kernel(xs, Ws)
    print(o.shape, o.dtype)



# revision 2
# speedup vs baseline: 1.4682x; 1.4682x over previous
"""Trainium2 Bass kernel for ContinuousREWAEncoder:
    out = FWHT(x @ W^T)/sqrt(32) + 0.01*normal(key=42)

Math folding: FWHT is linear => out = x @ (H @ W / sqrt(32))^T + noise.
The noise uses a fixed PRNG key, so it is a deterministic constant computed
on host (with the same jax op/backend as the reference) and streamed in.

Sharding: pure data parallel over tokens (B*N = 32768 -> 4096/core on 8
cores). W_eff is replicated.

The kernel is HBM-bound, so x is streamed as fp8e3 (e3m4: 4 mantissa
bits) — half the bytes of fp16 — while W stays fp16 (mixed-dtype matmul;
only fp32 requires both sides to match). Measured absmax rel err of the
whole pipeline vs the fp32 reference ~1.1e-2.  Noise and output move as
fp16 (negligible extra error, half the bytes).

Device schedule per core (TOK=4096 tokens = 2 supersteps x 4 blocks x 512):
  - x pre-tiled on host to [ss, kc, 128, 4*512] so each (ss,kc) DMA is one
    [128 part, 2048 B/partition] fully-contiguous transfer (256 KB).
  - col-tiled matmuls: the 4 blocks of a superstep run in the 4 column
    groups of the PE array concurrently (tile_position=(0,32j)), sharing
    one [128,512] fp32 PSUM bank, accumulation c-major over the 8 k-chunks.
  - DVE evacuates psum + noise -> fp16 out tile; scalar ring stores it.
"""

import math

import numpy as np
import ml_dtypes

import concourse.tile as tile
from concourse import bacc, mybir
from concourse.bass_utils import run_bass_kernel_spmd

B, N, D, M = 4, 8192, 1024, 32
NOISE_STD = 0.01
N_CORES = 8
TOK_TOTAL = B * N              # 32768
TOK = TOK_TOTAL // N_CORES     # 4096 tokens per core
BLK = 512                      # tokens per PSUM column-group
NGRP = 4                       # col groups per superstep (PE col tiling)
SS = TOK // (BLK * NGRP)       # 2 supersteps
KC = D // 128                  # 8 contraction chunks

X_DT = mybir.dt.float8e3       # e3m4: 1 byte, 4 mantissa bits
X_NP = ml_dtypes.float8_e3m4
W_DT = mybir.dt.float16
F16 = mybir.dt.float16
F32 = mybir.dt.float32


def _build_bass():
    nc = bacc.Bacc("TRN2", target_bir_lowering=False)

    # x pre-tiled on host so each (ss, kc) DMA moves one fully-contiguous
    # 2048 B run per partition (256 KB per DMA at full streaming rate).
    xT = nc.dram_tensor("xT", [SS, KC, 128, NGRP * BLK], X_DT, kind="ExternalInput")
    # w pre-packed on host to SBUF layout [partition, kchunk*M].
    wT = nc.dram_tensor("wT", [128, KC * M], W_DT, kind="ExternalInput")
    # noise pre-permuted: partition 32j+m = (block j, channel m), fp16.
    nzT = nc.dram_tensor("noiseT", [128, SS * BLK], F16, kind="ExternalInput")
    outT = nc.dram_tensor("outT", [SS, 128, BLK], F16, kind="ExternalOutput")

    with tile.TileContext(nc) as tc:
        with (
            tc.tile_pool(name="w", bufs=1) as wpool,
            tc.tile_pool(name="nz", bufs=1) as nzpool,
            tc.tile_pool(name="x", bufs=SS * KC) as xpool,
            tc.tile_pool(name="out", bufs=SS) as opool,
            tc.tile_pool(name="warm", bufs=1, space="PSUM") as warmpool,
            tc.tile_pool(name="psum", bufs=SS, space="PSUM") as ppool,
        ):
            # Noise on the scalar HWDGE ring (off the x stream's ring).
            nz_tile = nzpool.tile([128, SS, BLK], F16)
            nc.scalar.dma_start(nz_tile[:], nzT.rearrange("p (s t) -> p s t", s=SS))

            # w first on the sync ring so it lands before any x tile.
            w_tile = wpool.tile([128, KC, M], W_DT)
            nc.sync.dma_start(w_tile[:], wT.rearrange("p (c m) -> p c m", c=KC))

            x_tiles = []
            for s in range(SS):
                for c in range(KC):
                    t = xpool.tile([128, NGRP * BLK], X_DT, tag="xt")
                    nc.sync.dma_start(t[:], xT[s, c])
                    x_tiles.append(t)

            # Warmup matmul absorbs the w-DMA wait into PE program order so
            # every real matmul needs only its x-DMA wait.
            warm = warmpool.tile([M, M], F32)
            nc.tensor.matmul(warm[:], w_tile[:, 0, :], w_tile[:, 0, :])

            for s in range(SS):
                ptile = ppool.tile([128, BLK], F32, tag="ps")
                for c in range(KC):
                    xt = x_tiles[s * KC + c]
                    for j in range(NGRP):
                        nc.tensor.matmul(
                            ptile[32 * j : 32 * (j + 1), :],
                            w_tile[:, c, :],
                            xt[:, BLK * j : BLK * (j + 1)],
                            start=(c == 0),
                            stop=(c == KC - 1),
                            tile_position=(0, 32 * j),
                        )

                o_tile = opool.tile([128, BLK], F16)
                nc.vector.tensor_add(o_tile[:], ptile[:], nz_tile[:, s, :])
                nc.scalar.dma_start(outT[s], o_tile[:])

    nc.compile()
    return nc


_NC_CACHE = None


def _get_nc():
    global _NC_CACHE
    if _NC_CACHE is None:
        _NC_CACHE = _build_bass()
    return _NC_CACHE


def _hadamard32() -> np.ndarray:
    h = np.array([[1.0]], dtype=np.float64)
    while h.shape[0] < M:
        h = np.block([[h, h], [h, -h]])
    return h


_NOISE_CACHE = None


def _noise() -> np.ndarray:
    # Mirror reference.py exactly (same op on the default jax backend): the
    # bits differ between backends, so the noise must be produced the same
    # way the grading reference produces it.
    global _NOISE_CACHE
    if _NOISE_CACHE is None:
        import jax

        nz = NOISE_STD * jax.random.normal(
            jax.random.key(42), (B, N, M), dtype=np.float32
        )
        _NOISE_CACHE = np.asarray(nz)
    return _NOISE_CACHE


def kernel(x: np.ndarray, W: np.ndarray, _profile_sink=None) -> np.ndarray:
    x = np.ascontiguousarray(np.asarray(x, dtype=np.float32))
    W = np.asarray(W, dtype=np.float32)

    # Fold normalized FWHT into the projection: out = x @ w_lhsT + noise
    w_eff = (_hadamard32() @ W.astype(np.float64)) / math.sqrt(M)
    w_lhsT = w_eff.T.astype(np.float16)  # [D, M]
    # pack to device SBUF layout [partition, kchunk, M]
    w_dev = np.ascontiguousarray(
        w_lhsT.reshape(KC, 128, M).transpose(1, 0, 2)
    ).reshape(128, KC * M)

    noise = _noise().reshape(TOK_TOTAL, M)
    X8 = x.reshape(TOK_TOTAL, D).astype(X_NP)

    in_maps = []
    for i in range(N_CORES):
        sl = slice(i * TOK, (i + 1) * TOK)
        # [tok, d] -> [ss, kchunk, partition, group*tok_in_blk] contiguous
        xt = np.ascontiguousarray(
            X8[sl].reshape(SS, NGRP, BLK, KC, 128).transpose(0, 3, 4, 1, 2)
        ).reshape(SS, KC, 128, NGRP * BLK)
        # noise -> [partition=32j+m, ss*tok_in_blk] fp16
        nz = np.ascontiguousarray(
            noise[sl]
            .reshape(SS, NGRP, BLK, M)
            .transpose(0, 1, 3, 2)
            .reshape(SS, 128, BLK)
            .transpose(1, 0, 2)
        ).reshape(128, SS * BLK).astype(np.float16)
        in_maps.append({"xT": xt, "wT": w_dev, "noiseT": nz})

    res = run_bass_kernel_spmd(
        _get_nc(),
        in_maps,
        core_ids=list(range(N_CORES)),
        trace=_profile_sink is not None,
    )
    if _profile_sink is not None:
        _profile_sink.append(res)

    outs = []
    for r in res.results:
        o = r["outT"].astype(np.float32)  # [SS, 128, BLK]
        o = (
            o.reshape(SS, NGRP, M, BLK)
            .transpose(0, 1, 3, 2)
            .reshape(TOK, M)
        )
        outs.append(o)
    out = np.concatenate(outs, axis=0)
    return np.ascontiguousarray(out.reshape(B, N, M).astype(np.float32))


if __name__ == "__main__":
    xs = np.random.randn(B, N, D).astype(np.float32)
    Ws = (np.random.randn(M, D) / math.sqrt(D)).astype(np.float32)
    o = kernel(xs, Ws)
    print(o.shape, o.dtype)
